# revision 7
# baseline (speedup 1.0000x reference)
"""Multi-head attention (B=2, S=2048, D=1024, H=16) on 8 Trainium2 NeuronCores.

Sharding: core c -> batch b = c // 4, head-group g = c % 4 (4 heads = 256 proj
dims per core). Each core computes its 4 heads' attention plus the matching
slice of the output projection; the host sums the 4 partial outputs per batch
and adds bo.

v3 changes vs v2 (221us):
  - kc processed in PAIRS: the two row-tiled QK matmul pairs run back-to-back,
    then all full-array matmuls (PV/proj/emit) -- one PE tiling-mode switch
    per direction per kc-pair instead of per kc (each switch drains the
    array, ~90ns).
  - P@V stationary widened to [128,128]: cols 64-127 are ones, so PSUM rows
    64-127 hold the softmax denominator REPLICATED 64x. Normalize becomes
    reciprocal([64,512]) + one fused mul -- no 1-partition ops, no den/un
    copies, no rec16 cast, no gpsimd partition_broadcast (~6us critical
    latency -> ~2.4us, and ~25us of DVE work removed). Matmul cycles
    unchanged (cost = moving size).
  - bv folded into bo on the host: sum(p)/D == 1 makes the v-bias a constant
    shift of attn_out, which commutes through Wo. Removes 16 bias matmuls.
  - v_sb ones region filled by gpsimd memsets (DVE COPYs removed).
  - exp split rebalanced ACT:DVE = 10:6 per 16 kc.
  - tail: first 4 final-emit c0 matmuls issued before the last normalize
    completes; final PSUM->SBUF copies alternate DVE/ACT.
"""

import ml_dtypes
import numpy as np

import concourse.bass as bass
import concourse.mybir as mybir
import concourse.tile as tile
from concourse import bacc
from concourse.bass_utils import run_bass_kernel_spmd

B, S, D, H = 2, 2048, 1024, 16
OL = 256          # local projection dims (4 heads x 64)
NI = D // 128     # contraction chunks for projections
NK = S // 128     # key chunks
NQ = S // 512     # query blocks

# kc tiles whose exp runs whole-tile on VectorE (Schraudolph: one
# tensor_scalar mult+add writing int16 bits that bitcast to fp16) instead of
# ScalarE's native Exp; one per kc-pair for the first six pairs
DVE_KC = (1, 3, 5, 7, 9, 11)
# exp(0.125*x) ~= bitcast_fp16(int16(EXP_A*x + EXP_B))
EXP_A = 0.125 * 1.4426950408889634 * 1024.0
EXP_B = 15360.0 - 44.7

_CACHE = {}


def _build():
    DT = mybir.dt.float16
    F32 = mybir.dt.float32
    I16 = mybir.dt.int16
    AF = mybir.ActivationFunctionType
    MUL = mybir.AluOpType.mult
    ADD = mybir.AluOpType.add

    nc = bacc.Bacc("TRN2", target_bir_lowering=False, debug=False, num_devices=8)

    qt_d = nc.dram_tensor("qt", [D, S], DT, kind="ExternalInput").ap() \
        .rearrange("(c p) s -> p c s", p=128)
    kt_d = nc.dram_tensor("kt", [D, S], DT, kind="ExternalInput").ap() \
        .rearrange("(c p) s -> p c s", p=128)
    vt_d = nc.dram_tensor("vt", [D, S], DT, kind="ExternalInput").ap() \
        .rearrange("(c p) s -> p c s", p=128)
    wq_d = nc.dram_tensor("wqt", [D, OL], DT, kind="ExternalInput").ap() \
        .rearrange("(c p) o -> p c o", p=128)
    wk_d = nc.dram_tensor("wkt", [D, OL], DT, kind="ExternalInput").ap() \
        .rearrange("(c p) o -> p c o", p=128)
    wv_d = nc.dram_tensor("wvt", [D, OL], DT, kind="ExternalInput").ap() \
        .rearrange("(c p) o -> p c o", p=128)
    bq_d = nc.dram_tensor("bq2", [2, 128, 1], F32, kind="ExternalInput").ap()
    bk_d = nc.dram_tensor("bk2", [2, 128, 1], F32, kind="ExternalInput").ap()
    wo_d = nc.dram_tensor("wot", [OL, D], DT, kind="ExternalInput").ap() \
        .rearrange("(c p) o -> p c o", p=128)
    ones_d = nc.dram_tensor("ones1", [128, 4, 64], DT, kind="ExternalInput").ap()
    out_d = nc.dram_tensor("out_t", [D, S], DT, kind="ExternalOutput").ap() \
        .rearrange("(c p) s -> c p s", p=128)

    with tile.TileContext(nc) as tc:
        with (
            tc.tile_pool(name="per", bufs=1) as per,
            tc.tile_pool(name="pr", bufs=8) as pr,
            tc.tile_pool(name="sm", bufs=2) as sm,
            tc.tile_pool(name="ot", bufs=2) as ot,
            tc.tile_pool(name="osg", bufs=4) as osg,
            tc.tile_pool(name="pj", bufs=2, space="PSUM") as pj,
            tc.tile_pool(name="p1", bufs=2, space="PSUM") as p1,
            tc.tile_pool(name="px", bufs=2, space="PSUM") as px,
        ):
            # --- persistent tiles
            as_k = per.tile([128, NI, S], DT, tag="ak", name="ak")
            as_q = per.tile([128, NI, S], DT, tag="aq", name="aq")
            as_v = per.tile([128, NI, S], DT, tag="av", name="av")
            ws_k = per.tile([128, NI, OL], DT, tag="wk", name="wk")
            ws_q = per.tile([128, NI, OL], DT, tag="wq", name="wq")
            ws_v = per.tile([128, NI, OL], DT, tag="wv", name="wv")
            wo_sb = per.tile([128, 2, D], DT, tag="wo", name="wo")
            qt_sb = [per.tile([128, S], DT, tag=f"qt{m}", name=f"qt{m}")
                     for m in range(2)]
            kt_sb = [per.tile([128, S], DT, tag=f"kt{m}", name=f"kt{m}")
                     for m in range(2)]
            # [keys, head, 64 v-dims + 64 ones]; the ones columns make PSUM
            # rows 64-127 of the PV accumulation hold the softmax denominator
            # replicated across 64 partitions (vectorizes the normalize)
            v_sb = [per.tile([128, 4, 128], DT, tag=f"v{sc}", name=f"v{sc}")
                    for sc in range(NK)]
            bq_sb = [per.tile([128, 1], F32, tag=f"bq{m}", name=f"bq{m}")
                     for m in range(2)]
            bk_sb = [per.tile([128, 1], F32, tag=f"bk{m}", name=f"bk{m}")
                     for m in range(2)]

            # ones regions DMA'd on the idle gpsimd SWDGE ring (keeps the
            # sync ring free for the big input loads); disjoint from the
            # ACT v-copy region so no false deps
            for sc in range(NK):
                nc.gpsimd.dma_start(v_sb[sc][:, :, 64:128], ones_d)

            # HAM warmup: the PE idles ~6us waiting for the first kt
            # quarter; junk matmuls there lift the clock gate to 8/8 so
            # the real projection chains run at 2.4GHz instead of 1.2
            wsrc = per.tile([128, 512], DT, tag="wsrc", name="wsrc")
            nc.vector.memset(wsrc[:], 0.0)

            def warm_mms(n):
                for _ in range(n):
                    wps = pj.tile([128, 512], F32, tag="pj", name="warm")
                    nc.tensor.matmul(
                        wps[:], wsrc[:, 0:128], wsrc[:],
                        start=True, stop=True)

            # engines only come alive ~8us in; the first kt quarter lands
            # ~18us. 24 junk MMs bridge the whole window so the real
            # chains start at 2.4GHz
            warm_mms(24)

            # --- input loads, Sync HWDGE FIFO, strictly in consumption
            # order, 0.5-2MB per transfer. kt first (k-chains), then qt s0
            # (q-chains for query block 0), vt q0-q2; the tail (vt q3,
            # qt s1-3) streams from inside the attention loops.
            for m in range(2):
                nc.sync.dma_start(bq_sb[m][:], bq_d[m])
                nc.sync.dma_start(bk_sb[m][:], bk_d[m])
            nc.sync.dma_start(ws_k[:], wk_d)
            for j in range(4):
                nc.sync.dma_start(as_k[:, :, j * 512:(j + 1) * 512],
                                  kt_d[:, :, j * 512:(j + 1) * 512])
            nc.sync.dma_start(ws_q[:], wq_d)
            nc.sync.dma_start(as_q[:, :, 0:512], qt_d[:, :, 0:512])
            nc.sync.dma_start(ws_v[:], wv_d)
            nc.sync.dma_start(as_v[:, :, 0:512], vt_d[:, :, 0:512])
            nc.sync.dma_start(as_v[:, :, 512:1024], vt_d[:, :, 512:1024])
            nc.sync.dma_start(as_v[:, :, 1024:1536], vt_d[:, :, 1024:1536])
            nc.sync.dma_start(wo_sb[:], wo_d)

            def q_chain(ws, as_, bias_sb, dst_sb, m, s):
                acc = pj.tile([128, 512], F32, tag="pj", name="pj")
                for i in range(NI):
                    nc.tensor.matmul(
                        acc[:],
                        ws[:, i, m * 128:(m + 1) * 128],
                        as_[:, i, s * 512:(s + 1) * 512],
                        start=(i == 0),
                        stop=(i == NI - 1),
                    )
                nc.scalar.activation(
                    dst_sb[m][:, s * 512:(s + 1) * 512], acc[:],
                    AF.Identity, bias=bias_sb[m][:])

            def v_chain(sc):
                acc = pj.tile([128, OL], F32, tag="pj", name="pj")
                for i in range(NI):
                    nc.tensor.matmul(
                        acc[:],
                        as_v[:, i, sc * 128:(sc + 1) * 128],
                        ws_v[:, i, :],
                        start=(i == 0),
                        stop=(i == NI - 1),
                    )
                # ACT is stride-insensitive (1 elem/cycle); the strided
                # 4x64 dest costs DVE more but ACT only ~357ns
                nc.scalar.activation(
                    v_sb[sc][:, :, 0:64],
                    acc[:].rearrange("p (h d) -> p h d", h=4),
                    AF.Identity,
                )

            # prefix chains: kT (sg0/1 after kt half0, sg2/3 after half1),
            # then qT column s0. ACT is idle here so bias rides ACT.
            for sg in range(4):
                for m in range(2):
                    q_chain(ws_k, as_k, bk_sb, kt_sb, m, sg)
            for m in range(2):
                q_chain(ws_q, as_q, bq_sb, qt_sb, m, 0)

            # --- attention + output projection, per query block
            def emit_op(qb_, ots_src, oc, pool, tg, on_act=False):
                osl = slice(oc * 128, (oc + 1) * 128)
                pso = pool.tile([128, 512], F32, tag=tg, name="pso")
                for c in range(2):
                    nc.tensor.matmul(
                        pso[:], wo_sb[:, c, osl], ots_src[c][:],
                        start=(c == 0), stop=(c == 1),
                    )
                st = osg.tile([128, 512], DT, tag="st", name="st")
                if on_act:
                    nc.scalar.copy(st[:], pso[:])
                else:
                    nc.vector.tensor_copy(st[:], pso[:])
                # odd stores drain on the gpsimd SWDGE ring so two rings
                # pipeline the per-DMA fixed cost (matters for the tail)
                eng = nc.sync if oc % 2 == 0 else nc.gpsimd
                eng.dma_start(
                    out_d[oc][:, qb_ * 512:(qb_ + 1) * 512], st[:])

            ots_prev = None
            for qb in range(NQ):
                qsl = slice(qb * 512, (qb + 1) * 512)
                ots = [ot.tile([128, 512], DT, tag=f"c{c}", name=f"otc{c}")
                       for c in range(2)]
                for pair in range(2):
                    acc = [px.tile([128, 512], F32, tag="x", name="acc")
                           for _ in range(2)]
                    pend = []
                    op_iter = None
                    if pair == 0 and ots_prev is not None:
                        op_iter = iter(range(8))
                    for kc2 in range(NK // 2):
                        kcs = (2 * kc2, 2 * kc2 + 1)
                        # --- row-tiled section: both kc's QK pairs
                        ps1s = []
                        for kc in kcs:
                            ksl = slice(kc * 128, (kc + 1) * 128)
                            ps1 = p1.tile([128, 1024], F32, tag="s", name="s")
                            for hh in range(2):
                                psl = slice(hh * 64, (hh + 1) * 64)
                                nc.tensor.matmul(
                                    ps1[:, hh * 512:(hh + 1) * 512],
                                    kt_sb[pair][psl, ksl],
                                    qt_sb[pair][psl, qsl],
                                    start=True, stop=True,
                                )
                            ps1s.append(ps1)
                        # exp split across both engines (measured rates:
                        # ACT ~(172+n)/1.2, DVE ~(120+n)/0.96 ns)
                        for kc, ps1 in zip(kcs, ps1s):
                            prob = pr.tile([128, 1024], DT, tag="p", name="p")
                            if kc in DVE_KC:
                                nc.vector.tensor_scalar(
                                    out=prob[:].bitcast(I16), in0=ps1[:],
                                    scalar1=EXP_A, scalar2=EXP_B,
                                    op0=MUL, op1=ADD,
                                )
                            else:
                                nc.scalar.activation(
                                    prob[:], ps1[:], AF.Exp, scale=0.125
                                )
                            pend.append((kc, prob))
                        # --- full-array section: streams, PV, emits
                        if qb == 0 and pair == 0:
                            # stream vt q3 + v chains just in time
                            if kc2 == 0:
                                nc.sync.dma_start(
                                    as_v[:, :, 1536:2048],
                                    vt_d[:, :, 1536:2048])
                            for sc in (3 * kc2, 3 * kc2 + 1, 3 * kc2 + 2):
                                if sc < NK and kc2 < 6:
                                    v_chain(sc)
                        if pair == 1 and qb < NQ - 1:
                            # stream qt[s=qb+1] + its projection chains,
                            # two contraction steps (4 matmuls) per kc2
                            sN = qb + 1
                            if kc2 == 0:
                                nc.sync.dma_start(
                                    as_q[:, :, sN * 512:(sN + 1) * 512],
                                    qt_d[:, :, sN * 512:(sN + 1) * 512])
                            elif kc2 == 2:
                                qaccs = [pj.tile([128, 512], F32, tag="pj",
                                                 name="qacc")
                                         for _ in range(2)]
                            if 2 <= kc2 < 6:
                                for i in (2 * (kc2 - 2), 2 * (kc2 - 2) + 1):
                                    for m in range(2):
                                        nc.tensor.matmul(
                                            qaccs[m][:],
                                            ws_q[:, i, m * 128:(m + 1) * 128],
                                            as_q[:, i,
                                                 sN * 512:(sN + 1) * 512],
                                            start=(i == 0),
                                            stop=(i == NI - 1),
                                        )
                            elif kc2 == 6:
                                for m in range(2):
                                    nc.scalar.activation(
                                        qt_sb[m][:, sN * 512:(sN + 1) * 512],
                                        qaccs[m][:], AF.Identity,
                                        bias=bq_sb[m][:],
                                    )
                        while len(pend) > 2:
                            pkc, pprob = pend.pop(0)
                            for hh in range(2):
                                nc.tensor.matmul(
                                    acc[hh][:],
                                    v_sb[pkc][:, pair * 2 + hh, :],
                                    pprob[:, hh * 512:(hh + 1) * 512],
                                    start=(pkc == 0), stop=(pkc == NK - 1),
                                )
                        if op_iter is not None and kc2 >= 4:
                            for _ in range(2):
                                oc = next(op_iter, None)
                                if oc is not None:
                                    emit_op(qb - 1, ots_prev, oc, pj, "pj")
                    for pkc, pprob in pend:
                        for hh in range(2):
                            nc.tensor.matmul(
                                acc[hh][:], v_sb[pkc][:, pair * 2 + hh, :],
                                pprob[:, hh * 512:(hh + 1) * 512],
                                start=(pkc == 0), stop=(pkc == NK - 1),
                            )
                    if pair == 1:
                        # junk MMs keep HAM at 8/8 across the normalize
                        # latency so the next block's MMs run at full clock
                        warm_mms(4)
                    # normalize: PSUM rows 64-127 hold D replicated across
                    # 64 partitions. Copy D down to partitions 0-63 (plain
                    # copies handle the crossbar shift; the custom-DVE
                    # reciprocal does not on HW), then aligned reciprocal
                    # + one fused (PSUM x rec) mul.
                    for hh in range(2):
                        dsb = sm.tile([64, 512], F32, tag=f"d{hh}",
                                      name=f"d{hh}")
                        nc.vector.tensor_copy(dsb[:], acc[hh][64:128, :])
                        rec = sm.tile([64, 512], F32, tag=f"rc{hh}",
                                      name=f"rc{hh}")
                        nc.vector.reciprocal_approx_fast(rec[:], dsb[:])
                        nc.vector.tensor_mul(
                            ots[pair][hh * 64:(hh + 1) * 64, :],
                            acc[hh][0:64, :], rec[:],
                        )
                ots_prev = ots
            # final 8 emits: c0 matmuls for oc 0-3 first so the PE streams
            # while the last normalize chain finishes on DVE
            psos = []
            for oc in range(4):
                pool, tg = ((pj, "pj"), (px, "x"))[oc % 2]
                pso = pool.tile([128, 512], F32, tag=tg, name="pso")
                nc.tensor.matmul(
                    pso[:], wo_sb[:, 0, oc * 128:(oc + 1) * 128],
                    ots_prev[0][:], start=True, stop=False)
                psos.append(pso)
            for oc in range(4):
                pso = psos[oc]
                nc.tensor.matmul(
                    pso[:], wo_sb[:, 1, oc * 128:(oc + 1) * 128],
                    ots_prev[1][:], start=False, stop=True)
                st = osg.tile([128, 512], DT, tag="st", name="st")
                if oc % 2 == 1:
                    nc.scalar.copy(st[:], pso[:])
                else:
                    nc.vector.tensor_copy(st[:], pso[:])
                eng = nc.sync if oc % 2 == 0 else nc.gpsimd
                eng.dma_start(
                    out_d[oc][:, (NQ - 1) * 512:NQ * 512], st[:])
            for oc in range(4, 8):
                emit_op(NQ - 1, ots_prev, oc, (pj, px)[oc % 2],
                        ("pj", "x")[oc % 2], on_act=(oc % 2 == 1))

    nc.compile()
    return nc


def _get_nc():
    if "nc" not in _CACHE:
        _CACHE["nc"] = _build()
    return _CACHE["nc"]


def make_in_maps(Q, K, V, Wq, bq, Wk, bk, Wv, bv, Wo, bo):
    f = np.float32
    bf = np.float16
    in_maps = []
    for core in range(8):
        b, g = divmod(core, 4)
        sl = slice(g * OL, (g + 1) * OL)
        in_maps.append({
            "qt": np.ascontiguousarray(Q[b].T, dtype=bf),
            "kt": np.ascontiguousarray(K[b].T, dtype=bf),
            "vt": np.ascontiguousarray(V[b].T, dtype=bf),
            "wqt": np.ascontiguousarray(Wq[sl].T, dtype=bf),
            "wkt": np.ascontiguousarray(Wk[sl].T, dtype=bf),
            "wvt": np.ascontiguousarray(Wv[sl].T, dtype=bf),
            "bq2": np.ascontiguousarray(bq[sl].reshape(2, 128, 1), dtype=f),
            "bk2": np.ascontiguousarray(bk[sl].reshape(2, 128, 1), dtype=f),
            "wot": np.ascontiguousarray(Wo[:, sl].T, dtype=bf),
            "ones1": np.ones((128, 4, 64), dtype=bf),
        })
    return in_maps


def kernel(Q, K, V, Wq, bq, Wk, bk, Wv, bv, Wo, bo):
    nc = _get_nc()
    in_maps = make_in_maps(Q, K, V, Wq, bq, Wk, bk, Wv, bv, Wo, bo)
    res = run_bass_kernel_spmd(nc, in_maps, core_ids=list(range(8)))
    # sum(p)/D == 1 makes the v-bias a constant shift of attn_out, which
    # commutes through the output projection: fold bv into bo here.
    bo_eff = bo + Wo @ bv
    out = np.empty((B, S, D), np.float32)
    for b in range(B):
        acc = res.results[b * 4 + 0]["out_t"].astype(np.float32)
        for g in range(1, 4):
            acc += res.results[b * 4 + g]["out_t"]
        out[b] = (acc.T + bo_eff).astype(np.float32)
    return out
